# revision 45
# baseline (speedup 1.0000x reference)
"""Bahdanau attention Trainium2 Bass kernel.

Computes, for inputs decoder_hidden [B,H], encoder_outputs [B,S,H],
W1 [H,H], W2 [H,H], v [H] (B=64, S=1024, H=1024):

    dh_proj = decoder_hidden @ W1.T                    # [B, H]
    enc_proj = encoder_outputs @ W2.T                  # [B, S, H]
    energy = tanh(dh_proj[:, None, :] + enc_proj)      # [B, S, H]
    scores = energy @ v                                # [B, S]
    attn = softmax(scores, axis=-1)                    # [B, S]
    context = attn @ encoder_outputs (per batch)       # [B, H]
    returns (context, attn)

Sharding: batch dim across 8 cores (8 batches/core), weights replicated.

Per-core dataflow (single pass over enc):
  - Weights are pre-transposed on the host (pure layout prep) so W2T/W1T
    tiles land in SBUF with h on partitions, ready as matmul operands.
  - enc is loaded naturally ([s,h], s on partitions), transposed on the
    tensor engine via identity matmuls (fp32 has no DMA transpose), and
    fed as the moving operand of enc_projT = W2T_blk.T @ encT.
  - tanh is fused with the dh_proj add on ScalarE (bias is per-partition
    because projT has the output-feature dim on partitions).
  - scores = v . energyT via PE with v-blocks as stationary -> scores land
    with s on the free dim, so softmax is a plain free-dim reduction.
  - softmax skips max-subtraction (scores are O(5); exp is safe in fp32,
    matching jax softmax to fp rounding).
  - context = attnT @ enc_natural accumulates on PE from the same natural
    enc tiles already in SBUF; normalization by 1/Z folds into the final
    scale, so unnormalized exp scores can be used as they stream.
"""

import numpy as np

import concourse.tile as tile
from concourse import bacc, mybir
from concourse.bass_utils import run_bass_kernel_spmd
from concourse.masks import make_identity

F32 = mybir.dt.float32
F32R = mybir.dt.float32r
AF = mybir.ActivationFunctionType

P = 128  # partitions / PE tile size
N_CORES = 8


BF16 = mybir.dt.bfloat16


def build_nc(b_c=8, s=1024, h=1024, mm_dt=F32R, iters=1, ident_dt=None,
             ablate=(), tune=None, orient="b"):
    """Build the per-core Bass program. b_c batches/core, seq len s, hidden h.

    ablate: timing-ablation flags ("scores", "transpose", "dma") — produce
    WRONG numerics, used only to attribute hardware time between stages.
    """
    assert h == 1024 and s % 512 == 0
    HB = h // P          # h blocks (contraction)
    OB = h // P          # output-feature blocks
    n_sup = s // 512     # 512-row super tiles per batch
    n_chunk = s // P     # 128-row chunks per batch

    nc = bacc.Bacc("TRN2", target_bir_lowering=False, debug=False,
                   num_devices=N_CORES)

    enc = nc.dram_tensor("enc", [b_c * s, h], mm_dt, kind="ExternalInput").ap()
    w2t = nc.dram_tensor("w2t", [h, h], mm_dt, kind="ExternalInput").ap()
    w1t = nc.dram_tensor("w1t", [h, h], F32, kind="ExternalInput").ap()
    dht = nc.dram_tensor("dht", [h, b_c], F32, kind="ExternalInput").ap()
    vt = nc.dram_tensor("vt", [P, HB], mm_dt, kind="ExternalInput").ap()
    vrow = nc.dram_tensor("vrow", [1, h], F32, kind="ExternalInput").ap()
    ctx_out = nc.dram_tensor("ctx", [b_c, h], F32, kind="ExternalOutput").ap()
    attn_out = nc.dram_tensor("attn", [b_c, s], F32, kind="ExternalOutput").ap()
    attn_scat = [attn_out[b:b + 1, :].rearrange("one (c p) -> (one p) c", p=P)
                 for b in range(b_c)]

    with tile.TileContext(nc) as tc:
        from contextlib import ExitStack
        with ExitStack() as st:
            const_pool = st.enter_context(tc.tile_pool(name="const", bufs=1))
            identity_f32 = const_pool.tile([P, P], F32)
            make_identity(nc, identity_f32)
            id_dt = ident_dt if ident_dt is not None else mm_dt
            if id_dt == F32:
                identity = identity_f32
            else:
                identity = const_pool.tile([P, P], id_dt)
                nc.vector.tensor_copy(identity, identity_f32)
            one_t = const_pool.tile([1, 1], F32)
            nc.gpsimd.memset(one_t, 1.0)

            # ---- main pools
            tn = {"enc": n_chunk + 12, "encT": 16, "en": 6, "tp": 3, "pj": 2,
                  "sc": 1, "cx": 1, "rows": 2}
            if orient == "a":
                tn["enc"] = n_chunk + 7
                tn["encT"] = 14
            tn.update(tune or {})
            enc_pool = st.enter_context(tc.tile_pool(name="enc", bufs=tn["enc"]))

            # Startup ordering: prefetch the first enc super-tile so PE can
            # transpose while weights stream; weights load in o-halves (A =
            # o-blocks 0..3, B = 4..7) so the first projection matmuls start
            # after only half the weight bytes have landed.
            prefetched = {}
            for j in range(4):
                t = enc_pool.tile([P, h], mm_dt)
                nc.sync.dma_start(t, enc[j * P:(j + 1) * P, :])
                prefetched[j] = t

            vt_sb = const_pool.tile([P, HB], mm_dt)
            nc.gpsimd.dma_start(vt_sb, vt)
            dhp_pool = st.enter_context(tc.tile_pool(name="dhproj", bufs=OB))
            w2_pool = st.enter_context(tc.tile_pool(name="w2t", bufs=2 * HB))
            encT_pool = st.enter_context(tc.tile_pool(name="encT", bufs=tn["encT"]))
            tp_ps = st.enter_context(tc.tile_pool(name="tp_ps", bufs=tn["tp"], space="PSUM"))

            # transpose the prefetched first super-tile right away: this is
            # the only PE work available while the weight tiles stream in
            pre_encT = []
            if not ablate:
                for k in range(HB):
                    tp = tp_ps.tile([P, 512], mm_dt, tag="tp")
                    for j in range(4):
                        nc.tensor.matmul(
                            tp[:, j * P:(j + 1) * P],
                            lhsT=prefetched[j][:, k * P:(k + 1) * P],
                            rhs=identity, is_transpose=True,
                            start=(j == 0), stop=(j == 3))
                    e = encT_pool.tile([P, 512], mm_dt)
                    nc.vector.tensor_copy(e, tp)
                    pre_encT.append(e)

            hh = h // 2
            dh_projT = []
            w2t_half = [[], []]
            with tc.tile_pool(name="ph0", bufs=2 * HB) as ph0, \
                 tc.tile_pool(name="ph0ps", bufs=2, space="PSUM") as ph0ps:
                dht_sb = []
                for k in range(HB):
                    t = ph0.tile([P, b_c], F32, tag="dh", bufs=HB)
                    nc.gpsimd.dma_start(t, dht[k * P:(k + 1) * P, :])
                    dht_sb.append(t)
                w1t_half = [[], []]
                for half in range(2):
                    for k in range(HB):
                        t = ph0.tile([P, hh], F32, tag="w1")
                        nc.sync.dma_start(t, w1t[k * P:(k + 1) * P,
                                                 half * hh:(half + 1) * hh])
                        w1t_half[half].append(t)
                    for k in range(HB):
                        t = w2_pool.tile([P, hh], mm_dt, tag="w2")
                        nc.sync.dma_start(t, w2t[k * P:(k + 1) * P,
                                                 half * hh:(half + 1) * hh])
                        w2t_half[half].append(t)

                if orient == "b":
                    # dh_projT[o, b] = sum_h W1[o,h] dh[b,h], per o-block
                    for o in range(OB):
                        ps = ph0ps.tile([P, b_c], F32)
                        for k in range(HB):
                            nc.tensor.matmul(
                                ps,
                                lhsT=w1t_half[o // 4][k][:, (o % 4) * P:(o % 4 + 1) * P],
                                rhs=dht_sb[k],
                                start=(k == 0), stop=(k == HB - 1))
                        t = dhp_pool.tile([P, b_c], F32)
                        nc.vector.tensor_copy(t, ps)
                        dh_projT.append(t)
                else:
                    # natural dh_proj [b, o]: lhsT=dhT blocks, rhs=W1T halves
                    dh_proj = dhp_pool.tile([b_c, h], F32, tag="dhn")
                    for half in range(2):
                        ps = ph0ps.tile([b_c, hh], F32, tag="dhnps")
                        for k in range(HB):
                            nc.tensor.matmul(ps, lhsT=dht_sb[k],
                                             rhs=w1t_half[half][k],
                                             start=(k == 0), stop=(k == HB - 1))
                        nc.vector.tensor_copy(
                            dh_proj[:, half * hh:(half + 1) * hh], ps)

            def w2slice(k, o):
                return w2t_half[o // 4][k][:, (o % 4) * P:(o % 4 + 1) * P]

            if orient == "a":
                vrow_sb = const_pool.tile([1, h], F32)
                nc.sync.dma_start(vrow_sb, vrow)
                v_bcast = const_pool.tile([P, h], F32)
                nc.gpsimd.partition_broadcast(v_bcast, vrow_sb)
                ones_f32 = const_pool.tile([P, 1], F32)
                nc.gpsimd.memset(ones_f32, 1.0)
                ones_col = const_pool.tile([P, 1], mm_dt)
                nc.vector.tensor_copy(ones_col, ones_f32)
                dhb_pool = st.enter_context(tc.tile_pool(name="dhb", bufs=2))
                z_ps = st.enter_context(tc.tile_pool(name="z_ps", bufs=1,
                                                     space="PSUM"))
            en_pool = st.enter_context(tc.tile_pool(name="energy", bufs=tn["en"]))
            row_pool = st.enter_context(tc.tile_pool(name="rows", bufs=tn["rows"]))
            sm_pool = st.enter_context(tc.tile_pool(name="small", bufs=8))
            pj_ps = st.enter_context(tc.tile_pool(name="pj_ps", bufs=tn["pj"], space="PSUM"))
            sc_ps = st.enter_context(tc.tile_pool(name="sc_ps", bufs=tn["sc"], space="PSUM"))
            cx_ps = st.enter_context(tc.tile_pool(name="cx_ps", bufs=tn["cx"], space="PSUM"))

            first_tiles = []
            pending_flush = [None]  # deferred PE tail work of prev chunk/super
            for b in [bb for _ in range(iters) for bb in range(b_c)]:
                enc_tiles = []
                if orient == "b":
                    exp_row = row_pool.tile([1, s], F32, tag="exp")
                    zpart = sm_pool.tile([1, n_sup], F32, tag="z")
                if orient == "a":
                    dh_row = sm_pool.tile([1, h], F32, tag="dhr", bufs=1)
                    nc.sync.dma_start(dh_row, dh_proj[b:b + 1, :])
                    dhb = dhb_pool.tile([P, h], F32)
                    nc.gpsimd.partition_broadcast(dhb, dh_row)
                    exp_mat = sm_pool.tile([P, n_chunk], mm_dt, tag="expm")
                    zp = z_ps.tile([1, 1], F32)
                    cxp = cx_ps.tile([1, h], F32)
                for sup in range(n_sup):
                    # load 4 natural tiles [128, h]
                    for j in range(4):
                        if "dma" in ablate and len(first_tiles) >= 4:
                            enc_tiles.append(first_tiles[j])
                            continue
                        r0 = b * s + sup * 512 + j * P
                        pt = prefetched.pop(j, None) if (b, sup) == (0, 0) else None
                        if pt is not None:
                            enc_tiles.append(pt)
                            continue
                        t = enc_pool.tile([P, h], mm_dt)
                        nc.sync.dma_start(t, enc[r0:r0 + P, :])
                        enc_tiles.append(t)
                        if "dma" in ablate and len(first_tiles) < 4:
                            first_tiles.append(t)
                    # transpose to encT_k [h=128, r=512] per h-block
                    if pre_encT and (b, sup) == (0, 0):
                        encT = pre_encT
                        pre_encT = []
                    else:
                      encT = []
                      for k in range(HB):
                        if "transpose" in ablate:
                            e = encT_pool.tile([P, 512], mm_dt)
                            nc.vector.tensor_copy(
                                e, enc_tiles[sup * 4][:, 0:512])
                            encT.append(e)
                            continue
                        tp = tp_ps.tile([P, 512], mm_dt, tag="tp")
                        for j in range(4):
                            nc.tensor.matmul(
                                tp[:, j * P:(j + 1) * P],
                                lhsT=enc_tiles[sup * 4 + j][:, k * P:(k + 1) * P],
                                rhs=identity, is_transpose=True,
                                start=(j == 0), stop=(j == 3))
                        e = encT_pool.tile([P, 512], mm_dt)
                        nc.vector.tensor_copy(e, tp)
                        encT.append(e)
                    # flush the previous chunk/super's deferred PE work now
                    # that PE had transpose work to cover cross-engine latency
                    if pending_flush[0] is not None:
                        pending_flush[0]()
                        pending_flush[0] = None
                    if orient == "a":
                        # [r, o] orientation: dh-add on DVE (dh broadcast by
                        # GPSIMD), tanh on ACT, v-dot as fused DVE
                        # multiply-reduce along free dim; Z and context on PE
                        # from exp chunks, deferred one chunk for pipelining.
                        for rt in range(4):
                            ci = sup * 4 + rt
                            sc_p = [None]
                            for half in range(2):
                                pj = pj_ps.tile([P, 512], F32)
                                for k in range(HB):
                                    nc.tensor.matmul(
                                        pj,
                                        lhsT=encT[k][:, rt * P:(rt + 1) * P],
                                        rhs=w2t_half[half][k],
                                        start=(k == 0), stop=(k == HB - 1))
                                pre = en_pool.tile([P, 512], F32, tag="pre",
                                                   bufs=3)
                                nc.vector.tensor_add(
                                    pre, pj, dhb[:, half * hh:(half + 1) * hh])
                                ent = en_pool.tile([P, 512], F32, tag="ent",
                                                   bufs=3)
                                nc.scalar.activation(ent, pre, AF.Tanh)
                                junk = en_pool.tile([P, 512], F32, tag="junk",
                                                    bufs=2)
                                acc = sm_pool.tile([P, 1], F32, tag="sacc")
                                nc.vector.tensor_tensor_reduce(
                                    out=junk, in0=ent,
                                    in1=v_bcast[:, half * hh:(half + 1) * hh],
                                    scale=1.0,
                                    scalar=0.0 if half == 0 else sc_p[0],
                                    op0=mybir.AluOpType.mult,
                                    op1=mybir.AluOpType.add,
                                    accum_out=acc)
                                sc_p[0] = acc
                            nc.scalar.activation(exp_mat[:, ci:ci + 1],
                                                 sc_p[0], AF.Exp)

                            def zctx(ci=ci, et=enc_tiles[ci]):
                                nc.tensor.matmul(
                                    zp, lhsT=ones_col,
                                    rhs=exp_mat[:, ci:ci + 1],
                                    start=(ci == 0), stop=(ci == n_chunk - 1))
                                for hf in range(2):
                                    nc.tensor.matmul(
                                        cxp[:, hf * 512:(hf + 1) * 512],
                                        lhsT=exp_mat[:, ci:ci + 1],
                                        rhs=et[:, hf * 512:(hf + 1) * 512],
                                        start=(ci == 0),
                                        stop=(ci == n_chunk - 1))

                            if pending_flush[0] is not None:
                                pending_flush[0]()
                            pending_flush[0] = zctx
                        continue
                    # projT[o] = sum_k W2T[k][:,o].T @ encT[k]; energy; scores.
                    # The scores matmul for o is emitted after main matmuls of
                    # o+1 so PE never stalls waiting for tanh(o) on ACT.
                    scp = sc_ps.tile([1, 512], F32)
                    ens = []

                    def emit_score(o):
                        if "scores" not in ablate:
                            nc.tensor.matmul(scp, lhsT=vt_sb[:, o:o + 1],
                                             rhs=ens[o],
                                             start=(o == 0), stop=(o == OB - 1))
                        elif o == 0:
                            nc.tensor.matmul(scp, lhsT=vt_sb[:, o:o + 1],
                                             rhs=ens[o], start=True, stop=True)

                    for o in range(OB):
                        pj = pj_ps.tile([P, 512], F32)
                        for k in range(HB):
                            nc.tensor.matmul(
                                pj, lhsT=w2slice(k, o),
                                rhs=encT[k],
                                start=(k == 0), stop=(k == HB - 1))
                        en = en_pool.tile([P, 512], mm_dt)
                        nc.scalar.activation(en, pj, AF.Tanh,
                                             bias=dh_projT[o][:, b:b + 1])
                        ens.append(en)
                        if o >= 1:
                            emit_score(o - 1)

                    def flush(scp=scp, emit_score=emit_score, sup=sup,
                              exp_row=exp_row, zpart=zpart):
                        emit_score(OB - 1)
                        # exp + partial Z for this 512-wide chunk
                        nc.scalar.activation(
                            exp_row[:, sup * 512:(sup + 1) * 512],
                            scp, AF.Exp, accum_out=zpart[:, sup:sup + 1])

                    pending_flush[0] = flush

                # ---- batch epilogue: softmax normalize + context
                if pending_flush[0] is not None:
                    pending_flush[0]()
                    pending_flush[0] = None
                if orient == "a":
                    zs = sm_pool.tile([1, 1], F32, tag="zs")
                    nc.vector.tensor_copy(zs, zp)
                    invz = sm_pool.tile([1, 1], F32, tag="iz")
                    nc.vector.reciprocal(invz, zs)
                    invz_bc = sm_pool.tile([P, 1], F32, tag="izb")
                    nc.gpsimd.partition_broadcast(invz_bc, invz)
                    attn_mat = sm_pool.tile([P, n_chunk], F32, tag="am")
                    nc.vector.tensor_scalar_mul(attn_mat, exp_mat, invz_bc)
                    nc.sync.dma_start(
                        attn_scat[b], attn_mat)
                    ctx_row = row_pool.tile([1, h], F32, tag="ctx")
                    nc.vector.tensor_scalar_mul(ctx_row, cxp, invz)
                    nc.sync.dma_start(ctx_out[b:b + 1, :], ctx_row)
                    continue
                zsum = sm_pool.tile([1, 1], F32, tag="zs")
                nc.vector.tensor_reduce(zsum, zpart, axis=mybir.AxisListType.X,
                                        op=mybir.AluOpType.add)
                invz = sm_pool.tile([1, 1], F32, tag="iz")
                nc.vector.reciprocal(invz, zsum)
                attn_row = row_pool.tile([1, s], F32, tag="attn")
                nc.vector.tensor_scalar_mul(attn_row, exp_row, invz)
                nc.sync.dma_start(attn_out[b:b + 1, :], attn_row)

                # attnT chunks [s=128, 1] via K=1 matmuls (exp, unnormalized)
                atp = tp_ps.tile([P, n_chunk], F32, tag="tp")
                for cc in range(n_chunk):
                    nc.tensor.matmul(atp[:, cc:cc + 1],
                                     lhsT=exp_row[:, cc * P:(cc + 1) * P],
                                     rhs=one_t,
                                     start=(cc == 0), stop=(cc == n_chunk - 1))
                attnT = sm_pool.tile([P, n_chunk], mm_dt, tag="at")
                nc.vector.tensor_copy(attnT, atp)

                cxp = cx_ps.tile([1, h], F32)
                for cc in range(n_chunk):
                    for hf in range(2):
                        nc.tensor.matmul(
                            cxp[:, hf * 512:(hf + 1) * 512],
                            lhsT=attnT[:, cc:cc + 1],
                            rhs=enc_tiles[cc][:, hf * 512:(hf + 1) * 512],
                            start=(cc == 0), stop=(cc == n_chunk - 1))
                ctx_row = row_pool.tile([1, h], F32, tag="ctx")
                nc.vector.tensor_scalar_mul(ctx_row, cxp, invz)
                nc.sync.dma_start(ctx_out[b:b + 1, :], ctx_row)

    nc.compile()
    return nc


_NC_CACHE = {}


def _get_nc(b_c=8, s=1024, h=1024, mm_dt=F32R):
    key = (b_c, s, h, mm_dt)
    if key not in _NC_CACHE:
        _NC_CACHE[key] = build_nc(b_c, s, h, mm_dt)
    return _NC_CACHE[key]


def make_in_maps(decoder_hidden, encoder_outputs, W1, W2, v, n_cores=N_CORES):
    B, S, H = encoder_outputs.shape
    b_c = B // n_cores
    w1t = np.ascontiguousarray(np.asarray(W1, np.float32).T)
    w2t = np.ascontiguousarray(np.asarray(W2, np.float32).T)
    vt = np.ascontiguousarray(np.asarray(v, np.float32).reshape(H // P, P).T)
    in_maps = []
    for i in range(n_cores):
        sl = slice(i * b_c, (i + 1) * b_c)
        in_maps.append({
            "enc": np.ascontiguousarray(
                np.asarray(encoder_outputs[sl], np.float32).reshape(b_c * S, H)),
            "w1t": w1t,
            "w2t": w2t,
            "dht": np.ascontiguousarray(np.asarray(decoder_hidden[sl], np.float32).T),
            "vt": vt,
            "vrow": np.asarray(v, np.float32).reshape(1, H).copy(),
        })
    return in_maps


def kernel(decoder_hidden, encoder_outputs, W1, W2, v):
    decoder_hidden = np.asarray(decoder_hidden)
    encoder_outputs = np.asarray(encoder_outputs)
    B, S, H = encoder_outputs.shape
    b_c = B // N_CORES
    nc = _get_nc(b_c, S, H)
    in_maps = make_in_maps(decoder_hidden, encoder_outputs, W1, W2, v)
    res = run_bass_kernel_spmd(nc, in_maps, list(range(N_CORES)))
    context = np.concatenate([res.results[i]["ctx"] for i in range(N_CORES)], axis=0)
    attn = np.concatenate([res.results[i]["attn"] for i in range(N_CORES)], axis=0)
    return (context.astype(np.float32), attn.astype(np.float32))
